# revision 11
# baseline (speedup 1.0000x reference)
"""Sliding-window (radius-8, K=17) single-head attention along W.

Full problem: feature/position [2, 128, 64, 256] f32; 1x1 convs Wq/Wk (+bias)
produce q/k; scores over a 17-wide window along W; softmax (zero-padded
windows contribute exp(0)=1 to the denominator); output is the attn-weighted
sum of windows of x = feature + position.

Sharding: data-parallel over (B, H) — the 128 (b, h) rows are independent;
each of the 8 cores gets 16 rows, two per iteration.

Per row (x_row = [C=128, W=256], x held in bf16; measured end-to-end rel err
~6e-3 vs the fp32 reference):
  q = (Wq/sqrt(C)) x + bq/sqrt(C);  k = Wk x + bk     (bf16 PE matmuls, fp32
      PSUM accumulate, bias added during the f32r eviction)
  S^T[w', w] = k^T q   computed TRANSPOSED (keys on partitions) in f32r so
      exp sees near-fp32 scores and exp(S^T) lands in SBUF in the layout the
      den/out matmuls need.
  Band structure: key chunk 1 (keys 0..127) only reaches queries 0..135;
  chunk 2 (keys 128..255) only queries 120..255. exp / mask / den / out all
  operate on those 136-wide strips only (scores are computed full-width —
  f32r matmuls need >=256 moving cols for 1 cyc/row — but never read
  outside the strips). Masking is multiplicative 0/1 on exp(S) post-exp.
  den[w] (broadcast across partitions) = ones128.T @ att strips, PSUM-
      initialized by ones128.T @ (oobcount/128) for the zero-padded
      out-of-range taps (exp(0)=1 each); out = (x^T.T @ att) * recip(den).
  x^T chunks from bf16 PE transposes of x.

Schedule: two-stage software pipeline. Stage A(i) = x-add (gpsimd, bf16
out), q/k matmuls, x^T transposes (issued before the score matmuls so the
PE has dependency-free work while the bias evictions run on scalar), score
matmuls, one merged strip-exp (scalar), one merged strip-mask (vector),
x^T eviction (scalar). Stage B(i) = den matmuls (3), reciprocal (vector),
out matmuls (6), final normalize (vector), batched output DMA. Issue order
A(0), A(1), B(0), A(2), B(1), ... Input DMAs: rows 0-1 first, then one
bf16 constant blob, rows 2-3, rows 4-15, so compute starts early and the
sync queue issues few DMA instructions.
"""

import numpy as np
from contextlib import ExitStack

import concourse.bacc as bacc
import concourse.mybir as mybir
import concourse.tile as tile
from concourse.ap import AP
from concourse.bass_utils import run_bass_kernel_spmd

B, C, H, W = 2, 128, 64, 256
R = 8
NCORES = 8
ROWS = B * H // NCORES        # 16 (b, h) rows per core
CORES_PER_B = NCORES // B     # 4
F32 = mybir.dt.float32
F32R = mybir.dt.float32r
BF = mybir.dt.bfloat16
EXP = mybir.ActivationFunctionType.Exp
COPY = mybir.ActivationFunctionType.Copy
SW = 136                      # strip width: chunk1 queries [0:136), chunk2 [120:256)
A1 = 2 * W - SW               # chunk2 strip start within a row's 512 att cols (376)
CB = 1568                     # bf16 const blob cols: wq|wk|ident|ones|mask(544)|oob(512)


def apn(t, dims, off=0):
    v = t[:]
    return AP(v.tensor, v.offset + off, list(v.ap[:1]) + list(dims))


def build_nc():
    nc = bacc.Bacc(trn_type="TRN2")
    f_ext = nc.dram_tensor("feature", [C, ROWS, W], F32, kind="ExternalInput")
    p_ext = nc.dram_tensor("position", [C, ROWS, W], F32, kind="ExternalInput")
    cb_ext = nc.dram_tensor("constb", [C, CB], BF, kind="ExternalInput")
    cf_ext = nc.dram_tensor("constf", [C, 2], F32, kind="ExternalInput")
    out_ext = nc.dram_tensor("out", [C, ROWS, W], F32, kind="ExternalOutput")

    with tile.TileContext(nc) as tc, ExitStack() as ctx:
        const = ctx.enter_context(tc.tile_pool(name="const", bufs=1))
        inp = ctx.enter_context(tc.tile_pool(name="inp", bufs=3))

        blocks = {}   # iter -> (ft, pt, j): input tile pair + row offset

        def load_rows(r0, nrows, iters):
            ft = inp.tile([C, nrows, W], F32, tag="ft")
            nc.sync.dma_start(ft[:], f_ext[:, r0 : r0 + nrows, :])
            pt = inp.tile([C, nrows, W], F32, tag="pt")
            nc.sync.dma_start(pt[:], p_ext[:, r0 : r0 + nrows, :])
            for n, it in enumerate(iters):
                blocks[it] = (ft, pt, 2 * n)

        # rows 0-1 and the constant blob land first so compute starts early
        load_rows(0, 2, [0])
        cb = const.tile([C, CB], BF, tag="cb")
        nc.sync.dma_start(cb[:], cb_ext[:])
        cf = const.tile([C, 2], F32, tag="cf")
        nc.sync.dma_start(cf[:], cf_ext[:])
        load_rows(2, 2, [1])
        load_rows(4, 12, [2, 3, 4, 5, 6, 7])

        wq_t = cb[:, 0:128]
        wk_t = cb[:, 128:256]
        ident = cb[:, 256:384]
        ones_t = cb[:, 384:512]
        mask_v = apn(cb, [(2 * SW, 2), (SW, 2), (1, SW)], off=512)
        oob_v = cb[:, 1056:1568]
        bq_t = cf[:, 0:1]
        bk_t = cf[:, 1:2]

        # touch Exp once so the ACT table loads during the input-DMA ramp
        warm = const.tile([C, 1], F32, tag="warm")
        nc.scalar.activation(warm[:], bq_t, EXP)

        xp = ctx.enter_context(tc.tile_pool(name="x", bufs=2))
        qkp = ctx.enter_context(tc.tile_pool(name="qk", bufs=2))
        attp = ctx.enter_context(tc.tile_pool(name="att", bufs=2))
        sbT = ctx.enter_context(tc.tile_pool(name="sbT", bufs=2))
        rdp = ctx.enter_context(tc.tile_pool(name="rd", bufs=2))
        osp = ctx.enter_context(tc.tile_pool(name="os", bufs=2))
        psq = ctx.enter_context(tc.tile_pool(name="psq", bufs=1, space="PSUM"))
        psk = ctx.enter_context(tc.tile_pool(name="psk", bufs=1, space="PSUM"))
        pss = ctx.enter_context(tc.tile_pool(name="pss", bufs=1, space="PSUM"))
        psden = ctx.enter_context(tc.tile_pool(name="psden", bufs=1, space="PSUM"))
        psxt = ctx.enter_context(tc.tile_pool(name="psxt", bufs=1, space="PSUM"))
        pso = ctx.enter_context(tc.tile_pool(name="pso", bufs=2, space="PSUM"))

        NIT = ROWS // 2
        st = {}
        osb = {}

        def stageA(it):
            ft, pt, j = blocks[it]

            # x for both rows, downcast to bf16 in the add
            x2 = xp.tile([C, 2, W], BF, tag="x2")
            nc.gpsimd.tensor_add(x2[:], ft[:, j : j + 2, :], pt[:, j : j + 2, :])

            # q and k for both rows in one matmul each (512 moving cols)
            q_ps = psq.tile([C, 2 * W], F32, tag="q")
            nc.tensor.matmul(q_ps[:], wq_t, x2[:], start=True, stop=True)
            k_ps = psk.tile([C, 2 * W], F32, tag="k")
            nc.tensor.matmul(k_ps[:], wk_t, x2[:], start=True, stop=True)

            # x^T transposes depend only on x — keep the PE busy while the
            # q/k bias evictions run on scalar
            xt_ps = psxt.tile([C, 2 * W], BF, tag="xt")
            for rr in range(2):
                nc.tensor.transpose(
                    xt_ps[:, rr * W : rr * W + 128], x2[:, rr, 0:128], ident
                )
                nc.tensor.transpose(
                    xt_ps[:, rr * W + 128 : (rr + 1) * W], x2[:, rr, 128:256], ident
                )

            q_sb = qkp.tile([C, 2 * W], F32R, tag="q")
            nc.scalar.add(q_sb[:], q_ps[:], bq_t)
            k_sb = qkp.tile([C, 2 * W], F32R, tag="k")
            nc.scalar.add(k_sb[:], k_ps[:], bk_t)

            # scores per row, transposed: [C, 2, 512] across 2 PSUM banks
            s_ps = pss.tile([C, 2, 2 * W], F32, tag="s")
            for rr in range(2):
                q0 = rr * W
                nc.tensor.matmul(
                    s_ps[:, rr, 0:W],
                    k_sb[:, q0 : q0 + 128],
                    q_sb[:, q0 : q0 + W],
                    start=True, stop=True,
                )
                nc.tensor.matmul(
                    s_ps[:, rr, W : 2 * W],
                    k_sb[:, q0 + 128 : q0 + W],
                    q_sb[:, q0 : q0 + W],
                    start=True, stop=True,
                )
            # exp on the valid strips of both rows in one op
            att = attp.tile([C, 2, 2 * W], BF)
            strips = [(2 * W, 2), (A1, 2), (1, SW)]
            nc.scalar.activation(apn(att, strips), apn(s_ps, strips), EXP)
            # multiplicative 0/1 band mask, both rows in one op
            av = apn(att, strips)
            nc.vector.tensor_mul(av, av, mask_v)

            xT = sbT.tile([C, 2 * W], BF, tag="xT")
            nc.scalar.activation(xT[:], xt_ps[:], COPY)
            st[it] = (att, xT)

        def stageB(it):
            r = 2 * it
            att, xT = st.pop(it)
            # denominators, broadcast across partitions by the ones matmul;
            # PSUM-initialized with the oob counts (pre-divided by 128).
            den_ps = psden.tile([C, 2 * W], F32, tag="den")
            nc.tensor.matmul(den_ps[:], ones_t, oob_v, start=True, stop=False)
            nc.tensor.matmul(
                apn(den_ps, [(W, 2), (1, SW)]),
                ones_t,
                apn(att, [(2 * W, 2), (1, SW)]),
                start=False, stop=False,
            )
            nc.tensor.matmul(
                apn(den_ps, [(W, 2), (1, SW)], off=W - SW),
                ones_t,
                apn(att, [(2 * W, 2), (1, SW)], off=A1),
                start=False, stop=True,
            )
            rden = rdp.tile([C, 2 * W], F32)
            nc.vector.reciprocal_approx_fast(out=rden[:], in_=den_ps[:])

            o_ps = pso.tile([C, 2 * W], F32, tag="out")
            for rr in range(2):
                o0 = rr * W
                nc.tensor.matmul(
                    o_ps[:, o0 : o0 + SW],
                    xT[:, o0 : o0 + 128],
                    att[:, rr, 0:SW],
                    start=True, stop=False,
                )
                nc.tensor.matmul(
                    o_ps[:, o0 + W - SW : o0 + SW],
                    xT[:, o0 + 128 : o0 + W],
                    att[:, rr, A1 : A1 + 16],
                    start=False, stop=True,
                )
                nc.tensor.matmul(
                    o_ps[:, o0 + SW : o0 + W],
                    xT[:, o0 + 128 : o0 + W],
                    att[:, rr, A1 + 16 : 2 * W],
                    start=True, stop=True,
                )
            # final normalize into a 4-row output buffer; DMA every 2nd iter
            if it % 2 == 0:
                o_sb4 = osp.tile([C, 4, W], F32, tag="osb")
                osb[it // 2] = o_sb4
            o_sb = osb[it // 2]
            half = (it % 2) * 2
            nc.vector.tensor_mul(o_sb[:, half : half + 2, :], o_ps[:], rden[:])
            if it % 2 == 1:
                nc.sync.dma_start(out_ext[:, r - 2 : r + 2, :], o_sb[:])

        stageA(0)
        for it in range(1, NIT):
            stageA(it)
            stageB(it - 1)
        stageB(NIT - 1)

    nc.compile()
    return nc


def host_consts(Wq, bq, Wk, bk):
    import ml_dtypes

    sc = 1.0 / np.sqrt(np.float32(C))
    wqt = np.ascontiguousarray(Wq.astype(np.float32).T * sc)
    wkt = np.ascontiguousarray(Wk.astype(np.float32).T)
    ident = np.eye(C, dtype=np.float32)
    ones = np.ones((C, C), dtype=np.float32)

    # 0/1 band masks on the two valid strips (same for both rows):
    # chunk1: key p vs query w in [0, SW);  chunk2: key 128+p vs query 120+j
    maskc = np.zeros((C, 2, SW), dtype=np.float32)
    for p in range(C):
        for w in range(SW):
            if abs(p - w) <= R:
                maskc[p, 0, w] = 1.0
            if abs((128 + p) - (W - SW + w)) <= R:
                maskc[p, 1, w] = 1.0
    maskc = np.broadcast_to(maskc[:, None], (C, 2, 2, SW)).reshape(C, 4 * SW)

    # oob count per query w (pre-divided by 128: the ones-matmul sums over
    # 128 partitions), same row repeated on all partitions, two rows
    wgrid = np.arange(W)
    oob_row = (np.maximum(0, R - wgrid) + np.maximum(0, wgrid - (W - 1 - R))) / 128.0
    oob_bc = np.tile(oob_row.astype(np.float32), (C, 2))

    constb = np.concatenate(
        [wqt, wkt, ident, ones, maskc, oob_bc], axis=1
    ).astype(ml_dtypes.bfloat16)
    assert constb.shape == (C, CB), constb.shape
    constf = np.stack(
        [bq.astype(np.float32) * sc, bk.astype(np.float32)], axis=1
    ).reshape(C, 2)
    return np.ascontiguousarray(constb), np.ascontiguousarray(constf)


def core_inputs(feature, position, Wq, bq, Wk, bk):
    constb, constf = host_consts(Wq, bq, Wk, bk)
    in_maps = []
    for i in range(NCORES):
        b = i // CORES_PER_B
        h0 = (i % CORES_PER_B) * ROWS
        in_maps.append(
            {
                "feature": np.ascontiguousarray(
                    feature[b, :, h0 : h0 + ROWS, :], dtype=np.float32
                ),
                "position": np.ascontiguousarray(
                    position[b, :, h0 : h0 + ROWS, :], dtype=np.float32
                ),
                "constb": constb,
                "constf": constf,
            }
        )
    return in_maps


def kernel(feature, position, Wq, bq, Wk, bk):
    feature = np.asarray(feature, dtype=np.float32)
    position = np.asarray(position, dtype=np.float32)
    Wq = np.asarray(Wq, dtype=np.float32)
    bq = np.asarray(bq, dtype=np.float32)
    Wk = np.asarray(Wk, dtype=np.float32)
    bk = np.asarray(bk, dtype=np.float32)
    in_maps = core_inputs(feature, position, Wq, bq, Wk, bk)
    nc = build_nc()
    res = run_bass_kernel_spmd(nc, in_maps, list(range(NCORES)))
    out = np.empty((B, C, H, W), dtype=np.float32)
    for i in range(NCORES):
        b = i // CORES_PER_B
        h0 = (i % CORES_PER_B) * ROWS
        out[b, :, h0 : h0 + ROWS, :] = res.results[i]["out"]
    return out
